# revision 40
# baseline (speedup 1.0000x reference)
"""Trainium2 Bass kernel for a VAE-style AttnBlock.

Reference semantics (B=4, C=512, H=W=64, HW=4096):
    h   = GroupNorm32(x)                                  (fp32 stats)
    q/k/v = 1x1 conv(h)                                   (C x C weights)
    attn  = softmax(q^T k / sqrt(C)) over keys            (HW x HW per sample)
    out   = attn @ v
    y     = x + 1x1 conv(out)

Sharding: 8 cores = 4 samples x 2 query-halves. Each core gets its
sample's full x (spatially rotated so its query half sits in columns
[0:2048]) and computes GroupNorm + full K/V redundantly, queries /
attention / output projection for its 2048 columns. Attention is
permutation-equivariant over spatial positions, so the rotation is
exact. All cores run one SPMD program; only the data differs.

On-chip layout: channels on partitions ([C, HW] = 4 tiles of [128, *]).
The attention matrix is computed transposed (S^T[m, n] = k^T q) so the
PV product needs no transposes; softmax denominators come from a
ones-vector matmul accumulated alongside PV. exp() skips the max
subtraction: logits are ~N(0,1) after the 1/sqrt(C) scale (|max| < 7
measured), far inside fp32 range. Q/K/P/V are fp8e4m3 with DoubleRow
matmuls (contraction walks channel/key tile pairs); exp is shifted by
-3 so P fits fp8 range (the shift cancels in P/den). The residual path
stays fp32. Measured end-to-end relative error vs the fp32 reference
is ~2e-3.

Schedule notes (from NTFF profiles): dummy matmuls keep the PE's HAM
clock-gate warm through GroupNorm stats; stats are chunked so they
pipeline with the x DMA, with all four channel-tiles' stats emitted
before any normalize-apply (each engine's stream stays unblocked); the
apply is chunked and fused with the Q/K/V projection matmuls; softmax
normalization is deferred past the output projection (division by
den[n] commutes with the channel matmul); each query-chunk's tail is
emitted after the next chunk's first S^T tiles so the PE never idles
long enough to re-throttle.
"""

import sys

for _p in ("/opt/trn_rl_repo",):
    if _p not in sys.path:
        sys.path.insert(0, _p)

import numpy as np
import ml_dtypes

C = 512
HW = 4096
NQ = 2048  # queries per core
CT = 4  # channel tiles of 128
MT = 32  # m (key) tiles of 128
NCHUNK = 512  # free-dim chunk (one PSUM bank of fp32)
NUM_GROUPS = 32
GSIZE = C // NUM_GROUPS  # 16 channels per group
EPS = 1e-6
N_CORES = 8
WARMUP_MMS = 28  # upfront dummy matmuls covering init until the x DMA lands
PV_LAG = 4  # S^T tile-pairs emitted ahead of their PV consumers
S_FP8 = True  # fp8e4m3 Q/K + DoubleRow for the logits matmul
PV_FP8 = True  # fp8e4m3 P/V + DoubleRow for the PV matmul
QKPROJ_FP8 = True  # fp8e4m3 h/weights + DoubleRow for the Q/K projections
EXP_SHIFT = 3.0  # exp(s - shift): keeps P inside fp8e4m3 range; cancels in P/den

_compiled = None


def _build_program():
    import concourse.bacc as bacc
    import concourse.mybir as mybir
    import concourse.tile as tile

    f32 = mybir.dt.float32
    bf16 = mybir.dt.bfloat16
    fp8 = mybir.dt.float8e4
    qk_dt = fp8 if S_FP8 else bf16
    ALU = mybir.AluOpType
    ACT = mybir.ActivationFunctionType
    AX = mybir.AxisListType
    DR = mybir.MatmulPerfMode.DoubleRow

    nc = bacc.Bacc("TRN2", target_bir_lowering=False, debug=False, num_devices=N_CORES)

    x_d = nc.dram_tensor("x", [C, HW], f32, kind="ExternalInput").ap()
    w_d = {
        name: nc.dram_tensor(name, [C, C], bf16, kind="ExternalInput").ap()
        for name in ("wqT", "wkT", "wvT", "woT")
    }
    if QKPROJ_FP8:
        for name in ("wqT8", "wkT8"):
            w_d[name] = nc.dram_tensor(name, [C, C], fp8, kind="ExternalInput").ap()
    vec_d = {
        name: nc.dram_tensor(name, [C], f32, kind="ExternalInput").ap()
        for name in ("bq", "bk", "boeff", "gamma", "beta")
    }
    gind_d = nc.dram_tensor("gind", [128, 128], f32, kind="ExternalInput").ap()
    y_d = nc.dram_tensor("y", [C, NQ], f32, kind="ExternalOutput").ap()

    xr = x_d.rearrange("(t p) m -> p t m", p=128)
    yr = y_d.rearrange("(t p) n -> p t n", p=128)

    scale_attn = float(C) ** -0.5
    inv_gn = 1.0 / float(GSIZE * HW)
    NJ = HW // NCHUNK  # 8 chunks over keys
    NJQ = NQ // NCHUNK  # 4 chunks over queries

    with tile.TileContext(nc) as tc:
        with (
            tc.tile_pool(name="consts", bufs=1) as cp,
            tc.tile_pool(name="hpool", bufs=1) as hp,
            tc.tile_pool(name="qkv", bufs=1) as qkvp,
        ):
            # wq first: the warm-up matmuls below only depend on it.
            wq_sb = cp.tile([128, CT, C], bf16, name="wq_sb")
            nc.sync.dma_start(wq_sb[:], w_d["wqT"].rearrange("(t p) o -> p t o", p=128))

            h = hp.tile([128, CT, HW], bf16, name="h")
            if QKPROJ_FP8:
                h8 = hp.tile([128, CT, HW], fp8, name="h8")
            q_sb = qkvp.tile([128, CT, NQ], qk_dt, name="q_sb")
            k_sb = qkvp.tile([128, CT, HW], qk_dt, name="k_sb")
            vT_sb = qkvp.tile([128, MT, NCHUNK], fp8 if PV_FP8 else bf16,
                              name="vT_sb")

            # ---------------- GroupNorm + QKV (chunk-fused) ----------------
            # Stats for all tiles are emitted first (so no engine's in-order
            # stream blocks another tile's stats), then the normalize-apply is
            # chunked and fused with the Q/K/V projection matmuls: the PE
            # starts real work as soon as the first 512-column chunk of h
            # exists.
            with (
                tc.tile_pool(name="xpool", bufs=1) as xp,
                tc.tile_pool(name="gnwork", bufs=1) as gw,
                tc.tile_pool(name="gnscr", bufs=2) as gscr,
                tc.tile_pool(name="gnps", bufs=2, space="PSUM") as gnps,
                tc.tile_pool(name="warmps", bufs=1, space="PSUM") as wps,
                tc.tile_pool(name="p2ps", bufs=4, space="PSUM") as p2,
            ):
                x_sb = xp.tile([128, CT, HW], f32, name="x_sb")
                for t in range(CT):
                    for jj in range(NJ):
                        js = slice(jj * NCHUNK, (jj + 1) * NCHUNK)
                        nc.sync.dma_start(x_sb[:, t, js], xr[:, t, js])

                # PE warm-up: keeps the HAM activity window busy while the
                # DVE/ACT engines run GroupNorm stats. An upfront burst on a
                # memset tile covers init; the rest are paced by their data
                # dependency on the stats stream.
                warm = wps.tile([128, NCHUNK], f32, name="warm")
                wtmp = cp.tile([128, NCHUNK], bf16, name="wtmp")
                nc.vector.memset(wtmp[:], 0.5)
                for _ in range(WARMUP_MMS):
                    nc.tensor.matmul(
                        warm[:], wtmp[:, 0:128], wtmp[:], start=True, stop=True
                    )

                w_sb = {"wqT": wq_sb}
                for name in ("wkT", "wvT", "woT"):
                    wt = cp.tile([128, CT, C], bf16, name=f"{name}_sb")
                    nc.sync.dma_start(wt[:], w_d[name].rearrange("(t p) o -> p t o", p=128))
                    w_sb[name] = wt
                if QKPROJ_FP8:
                    for name in ("wqT8", "wkT8"):
                        wt = cp.tile([128, CT, C], fp8, name=f"{name}_sb")
                        nc.sync.dma_start(
                            wt[:], w_d[name].rearrange("(t p) o -> p t o", p=128)
                        )
                        w_sb[name] = wt
                v_sb = {}
                for name in ("bq", "bk", "boeff", "gamma", "beta"):
                    vt = cp.tile([128, CT], f32, name=f"{name}_sb")
                    nc.sync.dma_start(vt[:], vec_d[name].rearrange("(t p) -> p t", p=128))
                    v_sb[name] = vt
                gind_sb = cp.tile([128, 128], f32, name="gind_sb")
                nc.sync.dma_start(gind_sb[:], gind_d[:])
                # All-ones weight with M=128: the den matmul then broadcasts
                # the denominator into every PSUM partition for free (same
                # streaming cost and bank count as an M=1 output).
                ones_m = cp.tile([128, 2, 128] if PV_FP8 else [128, 128],
                                 fp8 if PV_FP8 else bf16, name="ones_m")
                nc.vector.memset(ones_m[:], 1.0)
                eps_sb = cp.tile([128, 1], f32, name="eps_sb")
                nc.vector.memset(eps_sb[:], EPS)
                shift_sb = cp.tile([128, 1], f32, name="shift_sb")
                nc.vector.memset(shift_sb[:], -EXP_SHIFT if PV_FP8 else 0.0)

                sums = gw.tile([128, CT, 2], f32, name="sums")
                sparts = gw.tile([128, CT, NJ], f32, name="sparts")
                sqp = gw.tile([128, CT, NJ], f32, name="sqp")
                stats = cp.tile([128, CT, 8], f32, name="stats")

                # -- stats, all tiles --
                for t in range(CT):
                    for jj in range(NJ):
                        js = slice(jj * NCHUNK, (jj + 1) * NCHUNK)
                        nc.vector.tensor_reduce(
                            sparts[:, t, jj : jj + 1], x_sb[:, t, js], axis=AX.X, op=ALU.add
                        )
                        scr = gscr.tile([128, NCHUNK], f32, name="scr", tag="scr")
                        nc.scalar.activation(
                            scr[:], x_sb[:, t, js], ACT.Square,
                            accum_out=sqp[:, t, jj : jj + 1],
                        )
                        # paced warm-up: depends on this chunk's reduce (DVE)
                        # or square (ACT), so the PE shows activity at the
                        # pace GN stats actually run
                        pacer = sparts if jj % 2 == 0 else sqp
                        nc.tensor.matmul(
                            warm[0:1, 0:256],
                            pacer[:, t, jj : jj + 1],
                            x_sb[:, t, jj * NCHUNK : jj * NCHUNK + 256],
                            start=True, stop=True,
                        )
                    nc.vector.tensor_reduce(
                        sums[:, t, 0:1], sparts[:, t, :], axis=AX.X, op=ALU.add
                    )
                    nc.vector.tensor_reduce(
                        sums[:, t, 1:2], sqp[:, t, :], axis=AX.X, op=ALU.add
                    )
                    gps = gnps.tile([128, 2], f32, name="gps", tag="gn")
                    nc.tensor.matmul(
                        gps[:], gind_sb[:], sums[:, t, :], start=True, stop=True
                    )
                    st = stats[:, t, :]
                    # mean, E[x^2], mean^2, var, sqrt(var+eps), rstd, scale, shift
                    nc.vector.tensor_scalar(st[:, 0:1], gps[:, 0:1], inv_gn, None, ALU.mult)
                    nc.vector.tensor_scalar(st[:, 1:2], gps[:, 1:2], inv_gn, None, ALU.mult)
                    nc.vector.tensor_tensor(st[:, 2:3], st[:, 0:1], st[:, 0:1], ALU.mult)
                    nc.vector.tensor_tensor(st[:, 3:4], st[:, 1:2], st[:, 2:3], ALU.subtract)
                    nc.scalar.activation(st[:, 4:5], st[:, 3:4], ACT.Sqrt, bias=eps_sb[:])
                    nc.vector.reciprocal(st[:, 5:6], st[:, 4:5])
                    nc.vector.tensor_tensor(
                        st[:, 6:7], st[:, 5:6], v_sb["gamma"][:, t : t + 1], ALU.mult
                    )
                    nc.vector.tensor_tensor(st[:, 2:3], st[:, 0:1], st[:, 6:7], ALU.mult)
                    nc.vector.tensor_tensor(
                        st[:, 7:8], v_sb["beta"][:, t : t + 1], st[:, 2:3], ALU.subtract
                    )
                    # paced warm-up across the stats tail
                    nc.tensor.matmul(
                        warm[0:1, 0:256],
                        st[:, 7:8],
                        x_sb[:, t, 0:256],
                        start=True, stop=True,
                    )

                # -- chunked apply fused with Q/K/V projections --
                def apply_chunk(t, jj):
                    js = slice(jj * NCHUNK, (jj + 1) * NCHUNK)
                    st = stats[:, t, :]
                    if (t + jj) % 2 == 0:
                        nc.scalar.activation(
                            h[:, t, js], x_sb[:, t, js], ACT.Identity,
                            bias=st[:, 7:8], scale=st[:, 6:7],
                        )
                        if QKPROJ_FP8:
                            nc.vector.tensor_copy(h8[:, t, js], h[:, t, js])
                    else:
                        nc.vector.tensor_scalar(
                            h[:, t, js], x_sb[:, t, js], st[:, 6:7], st[:, 7:8],
                            ALU.mult, ALU.add,
                        )
                        if QKPROJ_FP8:
                            nc.scalar.copy(h8[:, t, js], h[:, t, js])

                for jj in range(NJ):
                    js = slice(jj * NCHUNK, (jj + 1) * NCHUNK)
                    for t in range(CT):
                        apply_chunk(t, jj)
                        if jj == 0:
                            # paced warm-up across the first applies, bridging
                            # the stats tail to the first projection matmuls
                            nc.tensor.matmul(
                                warm[:],
                                h[:, t, 0:128],
                                h[:, t, 0:NCHUNK],
                                start=True, stop=True,
                            )
                    def qk_proj(ps, wname, o):
                        if QKPROJ_FP8:
                            for T in range(CT // 2):
                                nc.tensor.matmul(
                                    ps[:],
                                    w_sb[wname + "8"][:, 2 * T : 2 * T + 2,
                                                      o * 128 : (o + 1) * 128],
                                    h8[:, 2 * T : 2 * T + 2, js],
                                    start=(T == 0),
                                    stop=(T == CT // 2 - 1),
                                    perf_mode=DR,
                                )
                        else:
                            for t in range(CT):
                                nc.tensor.matmul(
                                    ps[:],
                                    w_sb[wname][:, t, o * 128 : (o + 1) * 128],
                                    h[:, t, js],
                                    start=(t == 0),
                                    stop=(t == CT - 1),
                                )

                    for o in range(CT):
                        ps = p2.tile([128, NCHUNK], f32, name="psk", tag="p2")
                        qk_proj(ps, "wkT", o)
                        nc.vector.tensor_scalar(
                            k_sb[:, o, js], ps[:],
                            v_sb["bk"][:, o : o + 1], None, ALU.add,
                        )
                    if jj < NJQ:
                        for o in range(CT):
                            ps = p2.tile([128, NCHUNK], f32, name="psq", tag="p2")
                            qk_proj(ps, "wqT", o)
                            nc.scalar.activation(
                                q_sb[:, o, js], ps[:], ACT.Identity,
                                bias=v_sb["bq"][:, o : o + 1],
                            )
                    for u in range(4 * jj, 4 * jj + 4):
                        ps = p2.tile([128, NCHUNK], f32, name="psv", tag="p2")
                        for t in range(CT):
                            nc.tensor.matmul(
                                ps[:],
                                h[:, t, u * 128 : (u + 1) * 128],
                                w_sb["wvT"][:, t, :],
                                start=(t == 0),
                                stop=(t == CT - 1),
                            )
                        nc.vector.tensor_copy(vT_sb[:, u, :], ps[:])

            # ------- attention + output projection (tail-overlapped) ----
            with (
                tc.tile_pool(name="sps", bufs=3, space="PSUM") as sp,
                tc.tile_pool(name="pvps", bufs=1, space="PSUM") as pvp,
                tc.tile_pool(name="w3", bufs=1) as w3,
                tc.tile_pool(name="ptp", bufs=7) as ptp,
                tc.tile_pool(name="iop", bufs=2) as iop,
            ):
                state = {}  # per-j: pv, den, pts, xres

                def head(j):
                    njs = slice(j * NCHUNK, (j + 1) * NCHUNK)
                    xres = iop.tile([128, CT, NCHUNK], f32, name="xres", tag="xres")
                    nc.sync.dma_start(xres[:], xr[:, :, njs])
                    for o in range(CT):
                        nc.vector.tensor_scalar(
                            xres[:, o, :], xres[:, o, :],
                            v_sb["boeff"][:, o : o + 1], None, ALU.add,
                        )
                    state[j] = {"xres": xres, "pts": [None] * MT}

                def alloc_pv(j):
                    state[j]["pv"] = [
                        pvp.tile([128, NCHUNK], f32, name=f"pv{o}", tag=f"pv{o}")
                        for o in range(CT)
                    ]
                    state[j]["den"] = pvp.tile([128, NCHUNK], f32, name="den", tag="den")

                def s_tile(j, u):
                    njs = slice(j * NCHUNK, (j + 1) * NCHUNK)
                    ssp = sp.tile([128, NCHUNK], f32, name="ssp", tag="s3")
                    if S_FP8:
                        # DoubleRow: contraction over (partition, pair) —
                        # the pair dim walks the two 128-channel tiles.
                        for T in range(CT // 2):
                            nc.tensor.matmul(
                                ssp[:],
                                k_sb[:, 2 * T : 2 * T + 2, u * 128 : (u + 1) * 128],
                                q_sb[:, 2 * T : 2 * T + 2, njs],
                                start=(T == 0),
                                stop=(T == CT // 2 - 1),
                                perf_mode=DR,
                            )
                    else:
                        for t in range(CT):
                            nc.tensor.matmul(
                                ssp[:],
                                k_sb[:, t, u * 128 : (u + 1) * 128],
                                q_sb[:, t, njs],
                                start=(t == 0),
                                stop=(t == CT - 1),
                            )
                    if PV_FP8:
                        if u % 2 == 0:
                            pt = ptp.tile([128, 2, NCHUNK], fp8, name="pt", tag="pt")
                            state[j]["pts"][u // 2] = pt
                        nc.scalar.activation(
                            state[j]["pts"][u // 2][:, u % 2, :], ssp[:],
                            ACT.Exp, scale=scale_attn, bias=shift_sb[:],
                        )
                    else:
                        pt = ptp.tile([128, NCHUNK], bf16, name="pt", tag="pt")
                        nc.scalar.activation(pt[:], ssp[:], ACT.Exp, scale=scale_attn)
                        state[j]["pts"][u] = pt

                NPAIR = MT // 2

                def emit_pv(j, uu):
                    # uu indexes pairs of key tiles when PV_FP8, else tiles
                    stj = state[j]
                    last = NPAIR - 1 if PV_FP8 else MT - 1
                    pm = DR if PV_FP8 else None
                    if PV_FP8:
                        lhs_v = lambda o: vT_sb[:, 2 * uu : 2 * uu + 2,
                                                o * 128 : (o + 1) * 128]
                    else:
                        lhs_v = lambda o: vT_sb[:, uu, o * 128 : (o + 1) * 128]
                    nc.tensor.matmul(
                        stj["den"][:], ones_m[:], stj["pts"][uu][:],
                        start=(uu == 0), stop=(uu == last), perf_mode=pm,
                    )
                    for o in range(CT):
                        nc.tensor.matmul(
                            stj["pv"][o][:], lhs_v(o), stj["pts"][uu][:],
                            start=(uu == 0), stop=(uu == last), perf_mode=pm,
                        )
                    stj["pts"][uu] = None

                # Tail work for chunk j, split into small actions that are
                # interleaved one-per-S^T-step into the next chunk's stream:
                # softmax normalization is deferred past the output projection
                # (dividing by den[n] commutes with the channel matmul), and
                # no two tail actions hold "s3" PSUM slots at the same time,
                # so the S^T pipeline of the next chunk never starves.
                actions = []

                def tail_start(j):
                    stj = state.pop(j)
                    njs = slice(j * NCHUNK, (j + 1) * NCHUNK)
                    out_sb = w3.tile([128, CT, NCHUNK], bf16, name="out_sb", tag="out")
                    for o in range(CT):
                        nc.vector.tensor_copy(out_sb[:, o, :], stj["pv"][o][:])
                    y_sb = iop.tile([128, CT, NCHUNK], f32, name="y_sb", tag="y")
                    recipb = w3.tile([128, NCHUNK], f32, name="recipb", tag="recipb")

                    def recip_step():
                        # den is already broadcast across partitions by the
                        # ones matmul; no PSUM slot or PE work needed here.
                        nc.vector.reciprocal(recipb[:], stj["den"][:])

                    def proj_step(o):
                        def go():
                            yps = sp.tile([128, NCHUNK], f32, name="yps", tag="s3")
                            for t in range(CT):
                                nc.tensor.matmul(
                                    yps[:],
                                    w_sb["woT"][:, t, o * 128 : (o + 1) * 128],
                                    out_sb[:, t, :],
                                    start=(t == 0),
                                    stop=(t == CT - 1),
                                )
                            nc.vector.tensor_tensor(
                                y_sb[:, o, :], yps[:], recipb[:], ALU.mult
                            )
                            nc.vector.tensor_tensor(
                                y_sb[:, o, :], y_sb[:, o, :], stj["xres"][:, o, :],
                                ALU.add,
                            )
                        return go

                    actions.append(recip_step)
                    for o in range(CT):
                        actions.append(proj_step(o))
                    actions.append(lambda: nc.sync.dma_start(yr[:, :, njs], y_sb[:]))

                assert PV_FP8, "flat pipeline assumes paired fp8 PV"
                pending = []

                def pop_one():
                    jj, pp = pending.pop(0)
                    if pp == 0:
                        alloc_pv(jj)
                    emit_pv(jj, pp)
                    if pp == NPAIR - 1:
                        tail_start(jj)

                for j in range(NJQ):
                    head(j)
                    for u in range(MT):
                        s_tile(j, u)
                        if u % 2 == 1:
                            pending.append((j, u // 2))
                            if len(pending) > PV_LAG:
                                pop_one()
                        if actions:
                            actions.pop(0)()
                while pending:
                    pop_one()
                while actions:
                    actions.pop(0)()

    nc.compile()
    return nc


def get_program():
    global _compiled
    if _compiled is None:
        _compiled = _build_program()
    return _compiled


def make_in_maps(x, gn_gamma, gn_beta, wq, bq, wk, bk, wv, bv, wo, bo):
    bf = ml_dtypes.bfloat16
    f8 = ml_dtypes.float8_e4m3
    shared = {
        "wqT": np.ascontiguousarray(wq.T).astype(bf),
        "wkT": np.ascontiguousarray(wk.T).astype(bf),
        "wqT8": np.ascontiguousarray(wq.T).astype(f8),
        "wkT8": np.ascontiguousarray(wk.T).astype(f8),
        "wvT": np.ascontiguousarray(wv.T).astype(bf),
        "woT": np.ascontiguousarray(wo.T).astype(bf),
        "bq": np.ascontiguousarray(bq, np.float32),
        "bk": np.ascontiguousarray(bk, np.float32),
        "boeff": (wo.astype(np.float64) @ bv.astype(np.float64) + bo).astype(np.float32),
        "gamma": np.ascontiguousarray(gn_gamma, np.float32),
        "beta": np.ascontiguousarray(gn_beta, np.float32),
        "gind": (np.arange(128)[:, None] // GSIZE == np.arange(128)[None, :] // GSIZE
                 ).astype(np.float32),
    }
    in_maps = []
    for core in range(N_CORES):
        b, half = core // 2, core % 2
        xs = np.asarray(x[b], np.float32).reshape(C, HW)
        if half:
            xs = np.concatenate([xs[:, NQ:], xs[:, :NQ]], axis=1)
        in_maps.append({"x": np.ascontiguousarray(xs), **shared})
    return in_maps


def assemble_output(results, B, Hdim, Wdim):
    y = np.empty((B, C, HW), np.float32)
    for core in range(N_CORES):
        b, half = core // 2, core % 2
        y[b, :, half * NQ : (half + 1) * NQ] = results[core]["y"]
    return y.reshape(B, C, Hdim, Wdim)


def kernel(**inputs):
    from concourse.bass_utils import run_bass_kernel_spmd

    x = np.asarray(inputs["x"])
    B, _, Hdim, Wdim = x.shape
    nc = get_program()
    in_maps = make_in_maps(**inputs)
    res = run_bass_kernel_spmd(nc, in_maps, core_ids=list(range(N_CORES)))
    return assemble_output(res.results, B, Hdim, Wdim)


if __name__ == "__main__":
    rng = np.random.default_rng(0)
    ins = {
        "x": rng.standard_normal((4, C, 64, 64), np.float32),
        "gn_gamma": np.ones(C, np.float32),
        "gn_beta": np.zeros(C, np.float32),
    }
    s = 1.0 / np.sqrt(C)
    for nm in ("q", "k", "v", "o"):
        ins[f"w{nm}"] = rng.standard_normal((C, C), np.float32).astype(np.float32) * s
        ins[f"b{nm}"] = np.zeros(C, np.float32)
    out = kernel(**ins)
    print("kernel ran, out shape", out.shape, out.dtype)


# revision 41
# speedup vs baseline: 1.0461x; 1.0461x over previous
"""Trainium2 Bass kernel for a VAE-style AttnBlock.

Reference semantics (B=4, C=512, H=W=64, HW=4096):
    h   = GroupNorm32(x)                                  (fp32 stats)
    q/k/v = 1x1 conv(h)                                   (C x C weights)
    attn  = softmax(q^T k / sqrt(C)) over keys            (HW x HW per sample)
    out   = attn @ v
    y     = x + 1x1 conv(out)

Sharding: 8 cores = 4 samples x 2 query-halves. Each core gets its
sample's full x (spatially rotated so its query half sits in columns
[0:2048]) and computes GroupNorm + full K/V redundantly, queries /
attention / output projection for its 2048 columns. Attention is
permutation-equivariant over spatial positions, so the rotation is
exact. All cores run one SPMD program; only the data differs.

On-chip layout: channels on partitions ([C, HW] = 4 tiles of [128, *]).
The attention matrix is computed transposed (S^T[m, n] = k^T q) so the
PV product needs no transposes; softmax denominators come from a
ones-vector matmul accumulated alongside PV. exp() skips the max
subtraction: logits are ~N(0,1) after the 1/sqrt(C) scale (|max| < 7
measured), far inside fp32 range. Q/K/P/V are fp8e4m3 with DoubleRow
matmuls (contraction walks channel/key tile pairs); exp is shifted by
-3 so P fits fp8 range (the shift cancels in P/den). The residual path
stays fp32. Measured end-to-end relative error vs the fp32 reference
is ~2e-3.

Schedule notes (from NTFF profiles): dummy matmuls keep the PE's HAM
clock-gate warm through GroupNorm stats; stats are chunked so they
pipeline with the x DMA, with all four channel-tiles' stats emitted
before any normalize-apply (each engine's stream stays unblocked); the
apply is chunked and fused with the Q/K/V projection matmuls; softmax
normalization is deferred past the output projection (division by
den[n] commutes with the channel matmul); each query-chunk's tail is
emitted after the next chunk's first S^T tiles so the PE never idles
long enough to re-throttle.
"""

import sys

for _p in ("/opt/trn_rl_repo",):
    if _p not in sys.path:
        sys.path.insert(0, _p)

import numpy as np
import ml_dtypes

C = 512
HW = 4096
NQ = 2048  # queries per core
CT = 4  # channel tiles of 128
MT = 32  # m (key) tiles of 128
NCHUNK = 512  # free-dim chunk (one PSUM bank of fp32)
NUM_GROUPS = 32
GSIZE = C // NUM_GROUPS  # 16 channels per group
EPS = 1e-6
N_CORES = 8
WARMUP_MMS = 28  # upfront dummy matmuls covering init until the x DMA lands
PV_LAG = 4  # S^T tile-pairs emitted ahead of their PV consumers
S_FP8 = True  # fp8e4m3 Q/K + DoubleRow for the logits matmul
PV_FP8 = True  # fp8e4m3 P/V + DoubleRow for the PV matmul
QKPROJ_FP8 = True  # fp8e4m3 h/weights + DoubleRow for the Q/K projections
EXP_SHIFT = 3.0  # exp(s - shift): keeps P inside fp8e4m3 range; cancels in P/den

_compiled = None


def _build_program():
    import concourse.bacc as bacc
    import concourse.mybir as mybir
    import concourse.tile as tile

    f32 = mybir.dt.float32
    bf16 = mybir.dt.bfloat16
    fp8 = mybir.dt.float8e4
    qk_dt = fp8 if S_FP8 else bf16
    ALU = mybir.AluOpType
    ACT = mybir.ActivationFunctionType
    AX = mybir.AxisListType
    DR = mybir.MatmulPerfMode.DoubleRow

    nc = bacc.Bacc("TRN2", target_bir_lowering=False, debug=False, num_devices=N_CORES)

    x_d = nc.dram_tensor("x", [C, HW], f32, kind="ExternalInput").ap()
    w_d = {
        name: nc.dram_tensor(name, [C, C], bf16, kind="ExternalInput").ap()
        for name in ("wqT", "wkT", "wvT", "woT")
    }
    if QKPROJ_FP8:
        for name in ("wqT8", "wkT8"):
            w_d[name] = nc.dram_tensor(name, [C, C], fp8, kind="ExternalInput").ap()
    vec_d = {
        name: nc.dram_tensor(name, [C], f32, kind="ExternalInput").ap()
        for name in ("bq", "bk", "boeff", "gamma", "beta")
    }
    gind_d = nc.dram_tensor("gind", [128, 128], f32, kind="ExternalInput").ap()
    y_d = nc.dram_tensor("y", [C, NQ], f32, kind="ExternalOutput").ap()

    xr = x_d.rearrange("(t p) m -> p t m", p=128)
    yr = y_d.rearrange("(t p) n -> p t n", p=128)

    scale_attn = float(C) ** -0.5
    inv_gn = 1.0 / float(GSIZE * HW)
    NJ = HW // NCHUNK  # 8 chunks over keys
    NJQ = NQ // NCHUNK  # 4 chunks over queries

    with tile.TileContext(nc) as tc:
        with (
            tc.tile_pool(name="consts", bufs=1) as cp,
            tc.tile_pool(name="hpool", bufs=1) as hp,
            tc.tile_pool(name="qkv", bufs=1) as qkvp,
        ):
            # wq first: the warm-up matmuls below only depend on it.
            wq_sb = cp.tile([128, CT, C], bf16, name="wq_sb")
            nc.sync.dma_start(wq_sb[:], w_d["wqT"].rearrange("(t p) o -> p t o", p=128))

            h = hp.tile([128, CT, HW], bf16, name="h")
            if QKPROJ_FP8:
                h8 = hp.tile([128, CT, HW], fp8, name="h8")
            q_sb = qkvp.tile([128, CT, NQ], qk_dt, name="q_sb")
            k_sb = qkvp.tile([128, CT, HW], qk_dt, name="k_sb")
            vT_sb = qkvp.tile([128, MT, NCHUNK], fp8 if PV_FP8 else bf16,
                              name="vT_sb")

            # ---------------- GroupNorm + QKV (chunk-fused) ----------------
            # Stats for all tiles are emitted first (so no engine's in-order
            # stream blocks another tile's stats), then the normalize-apply is
            # chunked and fused with the Q/K/V projection matmuls: the PE
            # starts real work as soon as the first 512-column chunk of h
            # exists.
            with (
                tc.tile_pool(name="xpool", bufs=1) as xp,
                tc.tile_pool(name="gnwork", bufs=1) as gw,
                tc.tile_pool(name="gnscr", bufs=2) as gscr,
                tc.tile_pool(name="gnps", bufs=2, space="PSUM") as gnps,
                tc.tile_pool(name="warmps", bufs=1, space="PSUM") as wps,
                tc.tile_pool(name="p2ps", bufs=4, space="PSUM") as p2,
            ):
                x_sb = xp.tile([128, CT, HW], f32, name="x_sb")
                for t in range(CT):
                    for jj in range(NJ):
                        js = slice(jj * NCHUNK, (jj + 1) * NCHUNK)
                        nc.sync.dma_start(x_sb[:, t, js], xr[:, t, js])

                # PE warm-up: keeps the HAM activity window busy while the
                # DVE/ACT engines run GroupNorm stats. An upfront burst on a
                # memset tile covers init; the rest are paced by their data
                # dependency on the stats stream.
                warm = wps.tile([128, NCHUNK], f32, name="warm")
                wtmp = cp.tile([128, NCHUNK], bf16, name="wtmp")
                nc.vector.memset(wtmp[:], 0.5)
                for _ in range(WARMUP_MMS):
                    nc.tensor.matmul(
                        warm[:], wtmp[:, 0:128], wtmp[:], start=True, stop=True
                    )

                w_sb = {"wqT": wq_sb}
                for name in ("wkT", "wvT", "woT"):
                    wt = cp.tile([128, CT, C], bf16, name=f"{name}_sb")
                    nc.sync.dma_start(wt[:], w_d[name].rearrange("(t p) o -> p t o", p=128))
                    w_sb[name] = wt
                if QKPROJ_FP8:
                    for name in ("wqT8", "wkT8"):
                        wt = cp.tile([128, CT, C], fp8, name=f"{name}_sb")
                        nc.sync.dma_start(
                            wt[:], w_d[name].rearrange("(t p) o -> p t o", p=128)
                        )
                        w_sb[name] = wt
                v_sb = {}
                for name in ("bq", "bk", "boeff", "gamma", "beta"):
                    vt = cp.tile([128, CT], f32, name=f"{name}_sb")
                    nc.sync.dma_start(vt[:], vec_d[name].rearrange("(t p) -> p t", p=128))
                    v_sb[name] = vt
                gind_sb = cp.tile([128, 128], f32, name="gind_sb")
                nc.sync.dma_start(gind_sb[:], gind_d[:])
                # All-ones weight with M=128: the den matmul then broadcasts
                # the denominator into every PSUM partition for free (same
                # streaming cost and bank count as an M=1 output).
                ones_m = cp.tile([128, 2, 128] if PV_FP8 else [128, 128],
                                 fp8 if PV_FP8 else bf16, name="ones_m")
                nc.vector.memset(ones_m[:], 1.0)
                eps_sb = cp.tile([128, 1], f32, name="eps_sb")
                nc.vector.memset(eps_sb[:], EPS)
                shift_sb = cp.tile([128, 1], f32, name="shift_sb")
                nc.vector.memset(shift_sb[:], -EXP_SHIFT if PV_FP8 else 0.0)

                sums = gw.tile([128, CT, 2], f32, name="sums")
                sparts = gw.tile([128, CT, NJ], f32, name="sparts")
                sqp = gw.tile([128, CT, NJ], f32, name="sqp")
                stats = cp.tile([128, CT, 8], f32, name="stats")

                # -- stats, all tiles --
                for t in range(CT):
                    for jj in range(NJ):
                        js = slice(jj * NCHUNK, (jj + 1) * NCHUNK)
                        nc.vector.tensor_reduce(
                            sparts[:, t, jj : jj + 1], x_sb[:, t, js], axis=AX.X, op=ALU.add
                        )
                        scr = gscr.tile([128, NCHUNK], f32, name="scr", tag="scr")
                        nc.scalar.activation(
                            scr[:], x_sb[:, t, js], ACT.Square,
                            accum_out=sqp[:, t, jj : jj + 1],
                        )
                        # paced warm-up: depends on this chunk's reduce (DVE)
                        # or square (ACT), so the PE shows activity at the
                        # pace GN stats actually run
                        pacer = sparts if jj % 2 == 0 else sqp
                        nc.tensor.matmul(
                            warm[0:1, 0:256],
                            pacer[:, t, jj : jj + 1],
                            x_sb[:, t, jj * NCHUNK : jj * NCHUNK + 256],
                            start=True, stop=True,
                        )
                for t in range(CT):
                    nc.vector.tensor_reduce(
                        sums[:, t, 0:1], sparts[:, t, :], axis=AX.X, op=ALU.add
                    )
                    nc.vector.tensor_reduce(
                        sums[:, t, 1:2], sqp[:, t, :], axis=AX.X, op=ALU.add
                    )
                    gps = gnps.tile([128, 2], f32, name="gps", tag="gn")
                    nc.tensor.matmul(
                        gps[:], gind_sb[:], sums[:, t, :], start=True, stop=True
                    )
                    st = stats[:, t, :]
                    # mean, E[x^2], mean^2, var, sqrt(var+eps), rstd, scale, shift
                    nc.vector.tensor_scalar(st[:, 0:1], gps[:, 0:1], inv_gn, None, ALU.mult)
                    nc.vector.tensor_scalar(st[:, 1:2], gps[:, 1:2], inv_gn, None, ALU.mult)
                    nc.vector.tensor_tensor(st[:, 2:3], st[:, 0:1], st[:, 0:1], ALU.mult)
                    nc.vector.tensor_tensor(st[:, 3:4], st[:, 1:2], st[:, 2:3], ALU.subtract)
                    nc.scalar.activation(st[:, 4:5], st[:, 3:4], ACT.Sqrt, bias=eps_sb[:])
                    nc.vector.reciprocal(st[:, 5:6], st[:, 4:5])
                    nc.vector.tensor_tensor(
                        st[:, 6:7], st[:, 5:6], v_sb["gamma"][:, t : t + 1], ALU.mult
                    )
                    nc.vector.tensor_tensor(st[:, 2:3], st[:, 0:1], st[:, 6:7], ALU.mult)
                    nc.vector.tensor_tensor(
                        st[:, 7:8], v_sb["beta"][:, t : t + 1], st[:, 2:3], ALU.subtract
                    )
                    # paced warm-up across the stats tail
                    nc.tensor.matmul(
                        warm[0:1, 0:256],
                        st[:, 7:8],
                        x_sb[:, t, 0:256],
                        start=True, stop=True,
                    )

                # -- chunked apply fused with Q/K/V projections --
                def apply_chunk(t, jj):
                    js = slice(jj * NCHUNK, (jj + 1) * NCHUNK)
                    st = stats[:, t, :]
                    if (t + jj) % 2 == 0:
                        nc.scalar.activation(
                            h[:, t, js], x_sb[:, t, js], ACT.Identity,
                            bias=st[:, 7:8], scale=st[:, 6:7],
                        )
                        if QKPROJ_FP8:
                            nc.vector.tensor_copy(h8[:, t, js], h[:, t, js])
                    else:
                        nc.vector.tensor_scalar(
                            h[:, t, js], x_sb[:, t, js], st[:, 6:7], st[:, 7:8],
                            ALU.mult, ALU.add,
                        )
                        if QKPROJ_FP8:
                            nc.scalar.copy(h8[:, t, js], h[:, t, js])

                for jj in range(NJ):
                    js = slice(jj * NCHUNK, (jj + 1) * NCHUNK)
                    for t in range(CT):
                        apply_chunk(t, jj)
                        if jj == 0:
                            # paced warm-up across the first applies, bridging
                            # the stats tail to the first projection matmuls
                            nc.tensor.matmul(
                                warm[:],
                                h[:, t, 0:128],
                                h[:, t, 0:NCHUNK],
                                start=True, stop=True,
                            )
                    def qk_proj(ps, wname, o):
                        if QKPROJ_FP8:
                            for T in range(CT // 2):
                                nc.tensor.matmul(
                                    ps[:],
                                    w_sb[wname + "8"][:, 2 * T : 2 * T + 2,
                                                      o * 128 : (o + 1) * 128],
                                    h8[:, 2 * T : 2 * T + 2, js],
                                    start=(T == 0),
                                    stop=(T == CT // 2 - 1),
                                    perf_mode=DR,
                                )
                        else:
                            for t in range(CT):
                                nc.tensor.matmul(
                                    ps[:],
                                    w_sb[wname][:, t, o * 128 : (o + 1) * 128],
                                    h[:, t, js],
                                    start=(t == 0),
                                    stop=(t == CT - 1),
                                )

                    for o in range(CT):
                        ps = p2.tile([128, NCHUNK], f32, name="psk", tag="p2")
                        qk_proj(ps, "wkT", o)
                        nc.vector.tensor_scalar(
                            k_sb[:, o, js], ps[:],
                            v_sb["bk"][:, o : o + 1], None, ALU.add,
                        )
                    if jj < NJQ:
                        for o in range(CT):
                            ps = p2.tile([128, NCHUNK], f32, name="psq", tag="p2")
                            qk_proj(ps, "wqT", o)
                            nc.scalar.activation(
                                q_sb[:, o, js], ps[:], ACT.Identity,
                                bias=v_sb["bq"][:, o : o + 1],
                            )
                    for u in range(4 * jj, 4 * jj + 4):
                        ps = p2.tile([128, NCHUNK], f32, name="psv", tag="p2")
                        for t in range(CT):
                            nc.tensor.matmul(
                                ps[:],
                                h[:, t, u * 128 : (u + 1) * 128],
                                w_sb["wvT"][:, t, :],
                                start=(t == 0),
                                stop=(t == CT - 1),
                            )
                        nc.vector.tensor_copy(vT_sb[:, u, :], ps[:])

            # ------- attention + output projection (tail-overlapped) ----
            with (
                tc.tile_pool(name="sps", bufs=3, space="PSUM") as sp,
                tc.tile_pool(name="pvps", bufs=1, space="PSUM") as pvp,
                tc.tile_pool(name="w3", bufs=1) as w3,
                tc.tile_pool(name="ptp", bufs=7) as ptp,
                tc.tile_pool(name="iop", bufs=2) as iop,
            ):
                state = {}  # per-j: pv, den, pts, xres

                def head(j):
                    njs = slice(j * NCHUNK, (j + 1) * NCHUNK)
                    xres = iop.tile([128, CT, NCHUNK], f32, name="xres", tag="xres")
                    nc.sync.dma_start(xres[:], xr[:, :, njs])
                    for o in range(CT):
                        nc.vector.tensor_scalar(
                            xres[:, o, :], xres[:, o, :],
                            v_sb["boeff"][:, o : o + 1], None, ALU.add,
                        )
                    state[j] = {"xres": xres, "pts": [None] * MT}

                def alloc_pv(j):
                    state[j]["pv"] = [
                        pvp.tile([128, NCHUNK], f32, name=f"pv{o}", tag=f"pv{o}")
                        for o in range(CT)
                    ]
                    state[j]["den"] = pvp.tile([128, NCHUNK], f32, name="den", tag="den")

                def s_tile(j, u):
                    njs = slice(j * NCHUNK, (j + 1) * NCHUNK)
                    ssp = sp.tile([128, NCHUNK], f32, name="ssp", tag="s3")
                    if S_FP8:
                        # DoubleRow: contraction over (partition, pair) —
                        # the pair dim walks the two 128-channel tiles.
                        for T in range(CT // 2):
                            nc.tensor.matmul(
                                ssp[:],
                                k_sb[:, 2 * T : 2 * T + 2, u * 128 : (u + 1) * 128],
                                q_sb[:, 2 * T : 2 * T + 2, njs],
                                start=(T == 0),
                                stop=(T == CT // 2 - 1),
                                perf_mode=DR,
                            )
                    else:
                        for t in range(CT):
                            nc.tensor.matmul(
                                ssp[:],
                                k_sb[:, t, u * 128 : (u + 1) * 128],
                                q_sb[:, t, njs],
                                start=(t == 0),
                                stop=(t == CT - 1),
                            )
                    if PV_FP8:
                        if u % 2 == 0:
                            pt = ptp.tile([128, 2, NCHUNK], fp8, name="pt", tag="pt")
                            state[j]["pts"][u // 2] = pt
                        nc.scalar.activation(
                            state[j]["pts"][u // 2][:, u % 2, :], ssp[:],
                            ACT.Exp, scale=scale_attn, bias=shift_sb[:],
                        )
                    else:
                        pt = ptp.tile([128, NCHUNK], bf16, name="pt", tag="pt")
                        nc.scalar.activation(pt[:], ssp[:], ACT.Exp, scale=scale_attn)
                        state[j]["pts"][u] = pt

                NPAIR = MT // 2

                def emit_pv(j, uu):
                    # uu indexes pairs of key tiles when PV_FP8, else tiles
                    stj = state[j]
                    last = NPAIR - 1 if PV_FP8 else MT - 1
                    pm = DR if PV_FP8 else None
                    if PV_FP8:
                        lhs_v = lambda o: vT_sb[:, 2 * uu : 2 * uu + 2,
                                                o * 128 : (o + 1) * 128]
                    else:
                        lhs_v = lambda o: vT_sb[:, uu, o * 128 : (o + 1) * 128]
                    nc.tensor.matmul(
                        stj["den"][:], ones_m[:], stj["pts"][uu][:],
                        start=(uu == 0), stop=(uu == last), perf_mode=pm,
                    )
                    for o in range(CT):
                        nc.tensor.matmul(
                            stj["pv"][o][:], lhs_v(o), stj["pts"][uu][:],
                            start=(uu == 0), stop=(uu == last), perf_mode=pm,
                        )
                    stj["pts"][uu] = None

                # Tail work for chunk j, split into small actions that are
                # interleaved one-per-S^T-step into the next chunk's stream:
                # softmax normalization is deferred past the output projection
                # (dividing by den[n] commutes with the channel matmul), and
                # no two tail actions hold "s3" PSUM slots at the same time,
                # so the S^T pipeline of the next chunk never starves.
                actions = []

                def tail_start(j):
                    stj = state.pop(j)
                    njs = slice(j * NCHUNK, (j + 1) * NCHUNK)
                    out_sb = w3.tile([128, CT, NCHUNK], bf16, name="out_sb", tag="out")
                    for o in range(CT):
                        nc.vector.tensor_copy(out_sb[:, o, :], stj["pv"][o][:])
                    y_sb = iop.tile([128, CT, NCHUNK], f32, name="y_sb", tag="y")
                    recipb = w3.tile([128, NCHUNK], f32, name="recipb", tag="recipb")

                    def recip_step():
                        # den is already broadcast across partitions by the
                        # ones matmul; no PSUM slot or PE work needed here.
                        nc.vector.reciprocal(recipb[:], stj["den"][:])

                    def proj_step(o):
                        def go():
                            yps = sp.tile([128, NCHUNK], f32, name="yps", tag="s3")
                            for t in range(CT):
                                nc.tensor.matmul(
                                    yps[:],
                                    w_sb["woT"][:, t, o * 128 : (o + 1) * 128],
                                    out_sb[:, t, :],
                                    start=(t == 0),
                                    stop=(t == CT - 1),
                                )
                            nc.vector.tensor_tensor(
                                y_sb[:, o, :], yps[:], recipb[:], ALU.mult
                            )
                            nc.vector.tensor_tensor(
                                y_sb[:, o, :], y_sb[:, o, :], stj["xres"][:, o, :],
                                ALU.add,
                            )
                        return go

                    actions.append(recip_step)
                    for o in range(CT):
                        actions.append(proj_step(o))
                    actions.append(lambda: nc.sync.dma_start(yr[:, :, njs], y_sb[:]))

                assert PV_FP8, "flat pipeline assumes paired fp8 PV"
                pending = []

                def pop_one():
                    jj, pp = pending.pop(0)
                    if pp == 0:
                        alloc_pv(jj)
                    emit_pv(jj, pp)
                    if pp == NPAIR - 1:
                        tail_start(jj)

                for j in range(NJQ):
                    head(j)
                    for u in range(MT):
                        s_tile(j, u)
                        if u % 2 == 1:
                            pending.append((j, u // 2))
                            if len(pending) > PV_LAG:
                                pop_one()
                        if actions:
                            actions.pop(0)()
                while pending:
                    pop_one()
                while actions:
                    actions.pop(0)()

    nc.compile()
    return nc


def get_program():
    global _compiled
    if _compiled is None:
        _compiled = _build_program()
    return _compiled


def make_in_maps(x, gn_gamma, gn_beta, wq, bq, wk, bk, wv, bv, wo, bo):
    bf = ml_dtypes.bfloat16
    f8 = ml_dtypes.float8_e4m3
    shared = {
        "wqT": np.ascontiguousarray(wq.T).astype(bf),
        "wkT": np.ascontiguousarray(wk.T).astype(bf),
        "wqT8": np.ascontiguousarray(wq.T).astype(f8),
        "wkT8": np.ascontiguousarray(wk.T).astype(f8),
        "wvT": np.ascontiguousarray(wv.T).astype(bf),
        "woT": np.ascontiguousarray(wo.T).astype(bf),
        "bq": np.ascontiguousarray(bq, np.float32),
        "bk": np.ascontiguousarray(bk, np.float32),
        "boeff": (wo.astype(np.float64) @ bv.astype(np.float64) + bo).astype(np.float32),
        "gamma": np.ascontiguousarray(gn_gamma, np.float32),
        "beta": np.ascontiguousarray(gn_beta, np.float32),
        "gind": (np.arange(128)[:, None] // GSIZE == np.arange(128)[None, :] // GSIZE
                 ).astype(np.float32),
    }
    in_maps = []
    for core in range(N_CORES):
        b, half = core // 2, core % 2
        xs = np.asarray(x[b], np.float32).reshape(C, HW)
        if half:
            xs = np.concatenate([xs[:, NQ:], xs[:, :NQ]], axis=1)
        in_maps.append({"x": np.ascontiguousarray(xs), **shared})
    return in_maps


def assemble_output(results, B, Hdim, Wdim):
    y = np.empty((B, C, HW), np.float32)
    for core in range(N_CORES):
        b, half = core // 2, core % 2
        y[b, :, half * NQ : (half + 1) * NQ] = results[core]["y"]
    return y.reshape(B, C, Hdim, Wdim)


def kernel(**inputs):
    from concourse.bass_utils import run_bass_kernel_spmd

    x = np.asarray(inputs["x"])
    B, _, Hdim, Wdim = x.shape
    nc = get_program()
    in_maps = make_in_maps(**inputs)
    res = run_bass_kernel_spmd(nc, in_maps, core_ids=list(range(N_CORES)))
    return assemble_output(res.results, B, Hdim, Wdim)


if __name__ == "__main__":
    rng = np.random.default_rng(0)
    ins = {
        "x": rng.standard_normal((4, C, 64, 64), np.float32),
        "gn_gamma": np.ones(C, np.float32),
        "gn_beta": np.zeros(C, np.float32),
    }
    s = 1.0 / np.sqrt(C)
    for nm in ("q", "k", "v", "o"):
        ins[f"w{nm}"] = rng.standard_normal((C, C), np.float32).astype(np.float32) * s
        ins[f"b{nm}"] = np.zeros(C, np.float32)
    out = kernel(**ins)
    print("kernel ran, out shape", out.shape, out.dtype)
